# revision 47
# baseline (speedup 1.0000x reference)
"""Trainium2 Bass kernel for nn_Attention_43963285242601.

GQA attention block: q/k/v projections + RoPE + causal attention + o_proj,
tensor-parallel over 8 NeuronCores.

Sharding (core c of 8):
  - q-heads 4c..4c+3 and kv-head c: Wq/Wk/Wv column (head) shards,
    attention fully local per head group.
  - o_proj sharded over Wo ROWS (output features): every core computes
    out[:, 512c:512c+512] and needs the full attention output, which is
    distributed via AllGathers (bf16) that overlap with remaining
    attention / o_proj compute.  The last head-pair's AllGather is split
    into token halves so the o_proj tail is not gated on one late
    collective.
  - host concatenates the 8 feature shards: no all-reduce needed.

v3: all-bf16 matmuls, Q resident in SBUF, causal mask on the DVE,
reciprocal_approx_fast, full-width RoPE, V transposed by the DMA
crossbar (no PE transposes), one shared 8-bank PSUM pool, and a
software-pipelined schedule: projection passes interleave attention
bb=0 (front-loaded so collectives fire early), o_proj bb=0 interleaves
attention bb=1, 3-deep score-buffer rotation keeps the PE dense in
attention-only stretches.
"""

import numpy as np

import concourse.bacc as bacc
import concourse.mybir as mybir
import concourse.tile as tile
from concourse.bass_utils import run_bass_kernel_spmd

F32 = mybir.dt.float32
F32R = mybir.dt.float32r
BF16 = mybir.dt.bfloat16
AF = mybir.ActivationFunctionType

N_CORES = 8
B, L = 2, 2048
N_HEADS, N_KV = 32, 8
HEAD_DIM = 128
D = N_HEADS * HEAD_DIM
THETA = 500000.0

EXP_BIAS = -8.0


def _rope_tables(t_all, l, dh):
    half = dh // 2
    inv = 1.0 / (THETA ** (np.arange(half, dtype=np.float64) * 2.0 / dh))
    pos = np.arange(t_all, dtype=np.float64) % l
    ang = inv[:, None] * pos[None, :]  # [half, T]
    cos = np.cos(ang)
    sin = np.sin(ang)
    cc = np.concatenate([cos, cos], 0).astype(np.float32)
    ss = np.concatenate([-sin, sin], 0).astype(np.float32)
    return cc, ss


def _build(n_cores=N_CORES, b=B, l=L, nh=N_HEADS, nkv=N_KV):
    dh = HEAD_DIM
    d = nh * dh
    t_all = b * l
    hpc = nh // n_cores  # q heads per core
    assert nkv == n_cores, "one kv head per core"
    mpc = d // n_cores  # o_proj output features per core
    kt_d = d // dh  # contraction tiles for projections
    ktl = l // 128  # key tiles per batch
    qg_n = l // 512  # 512-wide query groups per (batch, head)
    tg_n = t_all // 512  # 512-wide token groups for projections
    ksub = 4  # k-tiles per x subslab load
    assert kt_d % ksub == 0
    nsub = kt_d // ksub
    n_hp = hpc // 2  # head-pairs per core
    scale = dh ** -0.5

    nc = bacc.Bacc(
        "TRN2", target_bir_lowering=False, debug=False, num_devices=n_cores
    )

    xT = nc.dram_tensor("xT", [d, t_all], BF16, kind="ExternalInput").ap()
    wqT = nc.dram_tensor("wqT", [d, hpc * dh], BF16, kind="ExternalInput").ap()
    wkT = nc.dram_tensor("wkT", [d, dh], BF16, kind="ExternalInput").ap()
    wvT = nc.dram_tensor("wvT", [d, dh], BF16, kind="ExternalInput").ap()
    woT = nc.dram_tensor("woT", [d, mpc], BF16, kind="ExternalInput").ap()
    outT = nc.dram_tensor("outT", [mpc, t_all], F32, kind="ExternalOutput").ap()

    # compile-time constants
    cc_np, ss_np = _rope_tables(t_all, l, dh)
    mask_np = np.zeros((128, 4 * 512), dtype=np.float32)
    for j in range(4):
        k_idx = np.arange(128)[:, None]
        q_idx = np.arange(512)[None, :]
        mask_np[:, j * 512 : (j + 1) * 512] = (128 * j + k_idx <= q_idx).astype(
            np.float32
        )
    import ml_dtypes as _mld

    cc_c = nc.inline_tensor(cc_np, name="cc_c").ap()
    ss_c = nc.inline_tensor(ss_np, name="ss_c").ap()
    mask_c = nc.inline_tensor(
        mask_np.astype(_mld.bfloat16), name="mask_c"
    ).ap()
    ones_f_c = nc.inline_tensor(
        np.ones((128, 128), dtype=_mld.bfloat16), name="ones_f_c"
    ).ap()
    ident_c = nc.inline_tensor(np.eye(128, dtype=np.float32), name="ident_c").ap()

    with tile.TileContext(nc) as tc:
        with (
            tc.tile_pool(name="constp", bufs=1) as constp,
            tc.tile_pool(name="kvp", bufs=1) as kvp,
            tc.tile_pool(name="dramp", bufs=1, space="DRAM") as dramp,
            tc.tile_pool(name="mainps", bufs=1, space="PSUM") as mainps,
            tc.tile_pool(name="ppool", bufs=3) as ppool,
            tc.tile_pool(name="accp", bufs=2) as accp,
            tc.tile_pool(name="obf", bufs=2) as obf,
            tc.tile_pool(name="bsb", bufs=2) as bsb,
        ):
            bias_t = constp.tile([128, 1], F32, tag="bias_t")
            nc.vector.memset(bias_t[:], EXP_BIAS)
            masks = constp.tile([128, 4 * 512], BF16, tag="masks")
            ones_f = constp.tile([128, 128], BF16, tag="ones_f")

            K = kvp.tile([128, t_all], BF16, tag="Kres")  # rotated K^T
            Q = kvp.tile([128, hpc, t_all], BF16, tag="Qres")  # rotated Q^T
            Vn = kvp.tile([128, b, ktl, 128], BF16, tag="Vn")  # V

            # bounce / gathered DRAM tiles.  (1,1) is split into token
            # halves so its AllGathers can fire before all of bb=1's
            # attention is done.
            bounce = {}
            gathered = {}
            for bb in range(b):
                for hp in range(n_hp):
                    if bb == 1 and hp == 1:
                        continue
                    bounce[(bb, hp)] = dramp.tile(
                        [2 * dh, l], BF16, tag=f"bounce{bb}_{hp}",
                        name=f"bounce{bb}_{hp}",
                    )
                    gathered[(bb, hp)] = dramp.tile(
                        [n_cores * 2 * dh, l], BF16,
                        addr_space="Shared" if n_cores > 4 else "Local",
                        tag=f"gath{bb}_{hp}", name=f"gath{bb}_{hp}",
                    )
            for q in range(qg_n):
                bounce[(1, 1, q)] = dramp.tile(
                    [2 * dh, 512], BF16, tag=f"bounce11_{q}",
                    name=f"bounce11_{q}",
                )
                gathered[(1, 1, q)] = dramp.tile(
                    [n_cores * 2 * dh, 512], BF16,
                    addr_space="Shared" if n_cores > 4 else "Local",
                    tag=f"gath11_{q}", name=f"gath11_{q}",
                )

            wq_r = wqT.rearrange("(k p) m -> p k m", p=128)
            wk_r = wkT.rearrange("(k p) m -> p k m", p=128)
            wv_r = wvT.rearrange("(k p) m -> p k m", p=128)
            xT_r = xT.rearrange("(k p) t -> p k t", p=128)

            # PSUM tags: ppA0/ppA1 (projection passes / o_proj / V transp),
            # psp x2 [128,1024] (score pairs + borrowed softmax tail),
            # po x2 = 8 banks.
            def _ps(tag):
                bufs = 2 if tag in ("psp", "po") else 1
                width = 1024 if tag == "psp" else 512
                return mainps.tile([128, width], F32, tag=tag, name=tag, bufs=bufs)

            # ---------------- attention group ----------------------------

            def _bounce_dma(bb, h, g, ob):
                if bb == 1 and h >= 2:
                    tgt = bounce[(1, 1, g)]
                    nc.sync.dma_start(
                        tgt[(h % 2) * dh : (h % 2 + 1) * dh, :],
                        ob[:],
                    )
                else:
                    nc.sync.dma_start(
                        bounce[(bb, h // 2)][
                            (h % 2) * dh : (h % 2 + 1) * dh,
                            g * 512 : (g + 1) * 512,
                        ],
                        ob[:],
                    )

            def _attn_group_body(bb, h, g, psp_cycle=None):
                # key tiles processed in PAIRS: scores land in a 2-bank
                # [128,1024] PSUM tile, one exp covers both; PV is
                # pipelined one pair behind the scores.
                qoff = bb * l + g * 512
                nkt = 4 * g + 4
                po = _ps("po")
                acc = accp.tile([128, 512], F32, tag="acc", name="acc")
                acc_r = accp.tile([128, 512], BF16, tag="acc_r", name="acc_r")
                Ps = []  # P tile per pair
                for pr in range(nkt // 2):
                    psp = _ps("psp")
                    for half in range(2):
                        kt = 2 * pr + half
                        nc.tensor.matmul(
                            psp[:, half * 512 : (half + 1) * 512],
                            K[:, bb * l + kt * 128 : bb * l + (kt + 1) * 128],
                            Q[:, h, qoff : qoff + 512],
                            start=True,
                            stop=True,
                            skip_group_check=True,
                        )
                    P = ppool.tile([128, 1024], BF16, tag="P", name="P")
                    nc.scalar.activation(
                        P[:], psp[:], AF.Exp, scale=scale, bias=bias_t[:]
                    )
                    for half in range(2):
                        kt = 2 * pr + half
                        j = kt - 4 * g
                        if j >= 0:
                            nc.vector.tensor_mul(
                                P[:, half * 512 : (half + 1) * 512],
                                P[:, half * 512 : (half + 1) * 512],
                                masks[:, j * 512 : (j + 1) * 512],
                            )
                    if pr >= 1:
                        for half in range(2):
                            kt = 2 * (pr - 1) + half
                            nc.tensor.matmul(
                                po[:],
                                Vn[:, bb, kt, :],
                                Ps[pr - 1][:, half * 512 : (half + 1) * 512],
                                start=(kt == 0),
                                stop=False,
                                skip_group_check=True,
                            )
                    # denominator accumulation on the DVE
                    if pr == 0:
                        nc.vector.tensor_add(
                            acc[:], P[:, 0:512], P[:, 512:1024]
                        )
                    else:
                        nc.vector.tensor_add(acc[:], acc[:], P[:, 0:512])
                        if pr == nkt // 2 - 1:
                            nc.vector.tensor_add(
                                acc_r[:], acc[:], P[:, 512:1024]
                            )
                        else:
                            nc.vector.tensor_add(acc[:], acc[:], P[:, 512:1024])
                    Ps.append(P)
                for half in range(2):
                    kt = nkt - 2 + half
                    nc.tensor.matmul(
                        po[:],
                        Vn[:, bb, kt, :],
                        Ps[-1][:, half * 512 : (half + 1) * 512],
                        start=(kt == 0),
                        stop=(half == 1),
                        skip_group_check=True,
                    )

                def _tail():
                    # borrow half a scores-rotation PSUM tile for the
                    # broadcasted column sums
                    pm = _ps("psp")[:, 0:512]
                    nc.tensor.matmul(
                        pm, ones_f[:], acc_r[:],
                        start=True, stop=True, skip_group_check=True,
                    )
                    bs = bsb.tile([128, 512], F32, tag="bs", name="bs")
                    nc.vector.reciprocal_approx_fast(bs[:], pm)
                    ob = obf.tile([128, 512], BF16, tag="ob", name="ob")
                    nc.vector.tensor_mul(ob[:], po[:], bs[:])
                    _bounce_dma(bb, h, g, ob)

                return _tail

            def _fire_allgather(key):
                nc.gpsimd.collective_compute(
                    "AllGather",
                    mybir.AluOpType.bypass,
                    replica_groups=[list(range(n_cores))],
                    ins=[bounce[key].opt()],
                    outs=[gathered[key].opt()],
                )

            # ============ W1/W2: projections + attention bb=0 =============
            with (
                tc.tile_pool(name="wpool", bufs=1) as wpool,
                tc.tile_pool(name="xpool", bufs=1) as xpool,
                tc.tile_pool(name="ropet", bufs=2) as ropet,
            ):
                wq_sb = wpool.tile([128, kt_d, hpc * dh], BF16, tag="wq")
                wk_sb = wpool.tile([128, kt_d, dh], BF16, tag="wk")
                wv_sb = wpool.tile([128, kt_d, dh], BF16, tag="wv")
                cc_sb = wpool.tile([128, t_all], F32, tag="cc_sb")
                ss_sb = wpool.tile([128, t_all], F32, tag="ss_sb")
                VT = wpool.tile([128, t_all], BF16, tag="VTres")  # V^T



                def _rope(dst, psrc, toff):
                    ts_t = ropet.tile([128, 512], F32, tag="ts")
                    r1 = ropet.tile([128, 512], F32, tag="r1")
                    nc.scalar.activation(ts_t[0:64, :], psrc[64:128, :], AF.Copy)
                    nc.scalar.activation(ts_t[64:128, :], psrc[0:64, :], AF.Copy)
                    nc.vector.tensor_mul(r1[:], psrc[:], cc_sb[:, toff : toff + 512])
                    nc.vector.tensor_mul(ts_t[:], ts_t[:], ss_sb[:, toff : toff + 512])
                    nc.vector.tensor_add(dst, r1[:], ts_t[:])

                first_load = [True]
                xs_cur = {}

                def _xs_for(tg):
                    # one resident x slab per token group (two half-tiles so
                    # the next group's first half can load while the last
                    # pass still reads the second); x read from HBM once.
                    if tg in xs_cur:
                        return xs_cur[tg]
                    xs_cur.clear()
                    kh = kt_d // 2
                    halves = (
                        xpool.tile([128, kh, 512], BF16, tag="xsA", name="xsA"),
                        xpool.tile([128, kh, 512], BF16, tag="xsB", name="xsB"),
                    )
                    toff = tg * 512
                    for hf in range(2):
                        for sub in range(nsub // 2):
                            ks = slice(sub * ksub, (sub + 1) * ksub)
                            gs = slice(
                                hf * kh + sub * ksub, hf * kh + (sub + 1) * ksub
                            )
                            # W1 groups split across two DMA queues so the
                            # cold-start arrival rate keeps up with the PE
                            eng = (
                                nc.scalar
                                if (tg < tg_n // 2 and sub % 2 == 1)
                                else nc.sync
                            )
                            eng.dma_start(
                                halves[hf][:, ks, :],
                                xT_r[:, gs, toff : toff + 512],
                            )
                    xs_cur[tg] = halves
                    return halves

                def _proj_pass(tg, otiles, tags):
                    toff = tg * 512
                    banks = [_ps(tags[i]) for i in range(len(otiles))]
                    xs = _xs_for(tg)
                    if first_load[0]:
                        for rs in (
                            slice(0, ksub),
                            slice(ksub, 3 * ksub),
                            slice(3 * ksub, kt_d),
                        ):
                            nc.gpsimd.dma_start(wq_sb[:, rs, :], wq_r[:, rs, :])
                            nc.gpsimd.dma_start(wk_sb[:, rs, :], wk_r[:, rs, :])
                            nc.gpsimd.dma_start(wv_sb[:, rs, :], wv_r[:, rs, :])
                        first_load[0] = False
                    for sub in range(nsub):
                        for i, ot in enumerate(otiles):
                            if ot[0] == "q":
                                w_ap = wq_sb
                                msl = slice(ot[1] * dh, (ot[1] + 1) * dh)
                            elif ot[0] == "k":
                                w_ap = wk_sb
                                msl = slice(0, dh)
                            else:
                                w_ap = wv_sb
                                msl = slice(0, dh)
                            for k in range(ksub):
                                kt = sub * ksub + k
                                kh = kt_d // 2
                                nc.tensor.matmul(
                                    banks[i][:],
                                    w_ap[:, kt, msl],
                                    xs[kt // kh][:, kt % kh, :],
                                    start=(kt == 0),
                                    stop=(kt == kt_d - 1),
                                    skip_group_check=True,
                                )
                    for i, ot in enumerate(otiles):
                        if ot[0] == "q":
                            _rope(Q[:, ot[1], toff : toff + 512], banks[i][:], toff)
                        elif ot[0] == "k":
                            _rope(K[:, toff : toff + 512], banks[i][:], toff)
                        else:
                            nc.scalar.activation(
                                VT[:, toff : toff + 512], banks[i][:], AF.Copy
                            )
                            bbv, ktb = tg // (tg_n // b), (tg % (tg_n // b)) * 4
                            for kl in range(4):
                                nc.scalar.dma_start_transpose(
                                    Vn[:, bbv, ktb + kl, :],
                                    VT[:, toff + kl * 128 : toff + (kl + 1) * 128],
                                )

                # prime the first token group's x chunks on both queues,
                # then let the constants ride the ACT DMA queue behind them
                _xs_for(0)
                nc.scalar.dma_start(cc_sb[:], cc_c)
                nc.scalar.dma_start(ss_sb[:], ss_c)
                nc.scalar.dma_start(masks[:], mask_c)
                nc.scalar.dma_start(ones_f[:], ones_f_c)

                PASSES = (
                    [("q", 0), ("q", 1)],
                    [("q", 2), ("q", 3)],
                    [("k",), ("v",)],
                )
                TAGS_A2 = ["ppA0", "ppA1"]
                TAGS_B2 = ["po", "po"]

                # ---- W1: tg 0..3 dense, ping-pong PSUM tag sets ----
                pidx = 0
                for tg in range(tg_n // 2):
                    for ot in PASSES:
                        _proj_pass(tg, ot, TAGS_A2 if pidx % 2 == 0 else TAGS_B2)
                        pidx += 1

                # ---- W2: tg 4..7 + attention bb=0, front-loaded ----
                attn_iter = [(0, h, g) for h in range(hpc) for g in range(qg_n)]
                st = {"ai": 0, "tail": None}

                def _emit_attn_unit():
                    if st["ai"] >= len(attn_iter):
                        return
                    bb, h, g = attn_iter[st["ai"]]
                    st["ai"] += 1
                    tail = _attn_group_body(bb, h, g)
                    if st["tail"] is not None:
                        st["tail"]()
                    st["tail"] = tail
                    if g == qg_n - 1 and h % 2 == 1:
                        st["tail"]()
                        st["tail"] = None
                        _fire_allgather((bb, h // 2))

                pace = [3, 3, 2, 2, 2, 2, 1, 1, 0, 0, 0, 0]
                passes = []
                for tg in range(tg_n // 2, tg_n):
                    for ot in PASSES:
                        passes.append((tg, ot))
                for pi, (tg, otiles) in enumerate(passes):
                    # once attention units are exhausted, ping-pong with the
                    # po tags (free again) to decouple back-to-back passes
                    tags = TAGS_B2 if (pi >= 8 and pi % 2 == 1) else TAGS_A2
                    _proj_pass(tg, otiles, tags)
                    for _ in range(pace[pi]):
                        _emit_attn_unit()
                while st["ai"] < len(attn_iter):
                    _emit_attn_unit()
                if st["tail"] is not None:
                    st["tail"]()
                    st["tail"] = None

            # ============ W3/W4: attention bb=1 + o_proj ==================
            with (
                tc.tile_pool(name="wopool", bufs=1) as wopool,
                tc.tile_pool(name="ogpool", bufs=2) as ogpool,
                tc.tile_pool(name="outst", bufs=3) as outst,
            ):
                wo_sb = wopool.tile([128, kt_d, mpc], BF16, tag="wo")
                wo_r = woT.rearrange("(k p) m -> p k m", p=128)
                # chunk 0 first (ungated) so the first o_proj m-block can
                # start as soon as its og lands; rest follow on gpsimd
                nc.gpsimd.dma_start(
                    wo_sb[:, :, 0:dh], wo_r[:, :, 0:dh]
                )

                kt_map = []
                for hp in range(n_hp):
                    for c in range(n_cores):
                        for hl in range(2):
                            kt_map.append(4 * c + 2 * hp + hl)
                blk = n_cores * 2

                g_rs = {}
                og_tiles = {}

                def _g_view(key):
                    if key not in g_rs:
                        g_rs[key] = gathered[key][:].rearrange(
                            "(k p) t -> p k t", p=128
                        )
                    return g_rs[key]

                def _og_load(bb, tgl):
                    og = ogpool.tile([128, kt_d, 512], BF16, tag="og", name="og")
                    nc.gpsimd.dma_start(
                        og[:, 0:blk, :],
                        _g_view((bb, 0))[:, :, tgl * 512 : (tgl + 1) * 512],
                    )
                    if bb == 1:
                        src = _g_view((1, 1, tgl))
                        nc.gpsimd.dma_start(og[:, blk : 2 * blk, :], src[:, :, :])
                    else:
                        nc.gpsimd.dma_start(
                            og[:, blk : 2 * blk, :],
                            _g_view((0, 1))[:, :, tgl * 512 : (tgl + 1) * 512],
                        )
                    return og

                oproj_units = [
                    (bb, tgl, m)
                    for bb in range(b)
                    for tgl in range(l // 512)
                    for m in range(mpc // 128)
                ]
                oi = [0]

                def _ensure_og(idx):
                    bb, tgl, m = oproj_units[idx]
                    key = (bb, tgl)
                    if key not in og_tiles:
                        og_tiles[key] = _og_load(bb, tgl)
                    return og_tiles[key]

                def _emit_oproj_unit():
                    if oi[0] >= len(oproj_units):
                        return False
                    bb, tgl, m = oproj_units[oi[0]]
                    og = _ensure_og(oi[0])
                    if m == 0 and oi[0] + 4 < len(oproj_units):
                        _ensure_og(oi[0] + 4)
                    pp = _ps("ppA0" if (oi[0] % 2 == 0) else "ppA1")
                    for kt in range(kt_d):
                        nc.tensor.matmul(
                            pp[:],
                            wo_sb[:, kt_map[kt], m * 128 : (m + 1) * 128],
                            og[:, kt, :],
                            start=(kt == 0),
                            stop=(kt == kt_d - 1),
                            skip_group_check=True,
                        )
                    ot = outst.tile([128, 512], F32, tag="ot", name="ot")
                    nc.scalar.activation(ot[:], pp[:], AF.Copy)
                    nc.sync.dma_start(
                        outT[
                            m * 128 : (m + 1) * 128,
                            bb * l + tgl * 512 : bb * l + (tgl + 1) * 512,
                        ],
                        ot[:],
                    )
                    oi[0] += 1
                    return True

                _ensure_og(0)
                _ensure_og(4)
                for m in range(1, mpc // dh):
                    nc.gpsimd.dma_start(
                        wo_sb[:, :, m * dh : (m + 1) * dh],
                        wo_r[:, :, m * dh : (m + 1) * dh],
                    )

                # bb=1 attention order: hp0's heads first (fires its full
                # AllGather), then hp1 g-major so its AllGathers fire per
                # 512-token quarter and the o_proj tail drains pipelined.
                attn_iter2 = (
                    [(1, 0, g) for g in range(qg_n)]
                    + [(1, 1, g) for g in range(qg_n)]
                    + [(1, h, g) for g in range(qg_n) for h in (2, 3)]
                )
                CYC3 = ["psp", "ppA2", "psp"]
                pending = None
                for u, (bb, h, g) in enumerate(attn_iter2):
                    tail = _attn_group_body(bb, h, g, CYC3)
                    if pending is not None:
                        pending()
                    pending = tail
                    if u == 7:
                        pending()
                        pending = None
                        _fire_allgather((1, 0))
                    elif u >= 9 and (u - 9) % 2 == 0:
                        pending()
                        pending = None
                        _fire_allgather((1, 1, (u - 9) // 2))
                    if u >= 4:
                        _emit_oproj_unit()
                    if u >= 12:
                        _emit_oproj_unit()
                if pending is not None:
                    pending()
                while _emit_oproj_unit():
                    pass

    nc.compile()
    return nc


_NC_CACHE = {}


def _get_nc(key=(N_CORES, B, L, N_HEADS, N_KV)):
    if key not in _NC_CACHE:
        _NC_CACHE[key] = _build(*key)
    return _NC_CACHE[key]


def make_in_maps(x, Wq, Wk, Wv, Wo, n_cores=N_CORES):
    import ml_dtypes

    bf16 = ml_dtypes.bfloat16
    b, l, d = x.shape
    nh = Wq.shape[0] // HEAD_DIM
    hpc = nh // n_cores
    mpc = d // n_cores
    xT = np.ascontiguousarray(x.reshape(b * l, d).T.astype(bf16))
    in_maps = []
    for c in range(n_cores):
        wq_c = np.ascontiguousarray(
            Wq[c * hpc * HEAD_DIM : (c + 1) * hpc * HEAD_DIM, :].T.astype(bf16)
        )
        wk_c = np.ascontiguousarray(
            Wk[c * HEAD_DIM : (c + 1) * HEAD_DIM, :].T.astype(bf16)
        )
        wv_c = np.ascontiguousarray(
            Wv[c * HEAD_DIM : (c + 1) * HEAD_DIM, :].T.astype(bf16)
        )
        wo_c = np.ascontiguousarray(
            Wo[c * mpc : (c + 1) * mpc, :].T.astype(bf16)
        )
        in_maps.append(
            {"xT": xT, "wqT": wq_c, "wkT": wk_c, "wvT": wv_c, "woT": wo_c}
        )
    return in_maps


def assemble_out(results, b, l, d):
    parts = [r["outT"] for r in results]
    outT = np.concatenate(parts, axis=0)  # [D, T]
    return np.ascontiguousarray(outT.T).reshape(b, l, d).astype(np.float32)


def kernel(x, Wq, Wk, Wv, Wo, trace=False):
    x = np.asarray(x, dtype=np.float32)
    nc = _get_nc()
    in_maps = make_in_maps(x, Wq, Wk, Wv, Wo)
    res = run_bass_kernel_spmd(nc, in_maps, list(range(N_CORES)), trace=trace)
    out = assemble_out(res.results, *x.shape)
    if trace:
        return out, res
    return out


if __name__ == "__main__":
    rng = np.random.default_rng(0)
    s = 0.02
    x = rng.standard_normal((B, L, D)).astype(np.float32)
    Wq = (rng.standard_normal((D, D)) * s).astype(np.float32)
    Wk = (rng.standard_normal((N_KV * HEAD_DIM, D)) * s).astype(np.float32)
    Wv = (rng.standard_normal((N_KV * HEAD_DIM, D)) * s).astype(np.float32)
    Wo = (rng.standard_normal((D, D)) * s).astype(np.float32)
    out = kernel(x, Wq, Wk, Wv, Wo)
    print(out.shape, out.dtype)


# revision 49
# speedup vs baseline: 1.1090x; 1.1090x over previous
"""Trainium2 Bass kernel for nn_Attention_43963285242601.

GQA attention block: q/k/v projections + RoPE + causal attention + o_proj,
tensor-parallel over 8 NeuronCores.

Sharding (core c of 8):
  - q-heads 4c..4c+3 and kv-head c: Wq/Wk/Wv column (head) shards,
    attention fully local per head group.
  - o_proj sharded over Wo ROWS (output features): every core computes
    out[:, 512c:512c+512] and needs the full attention output, which is
    distributed via AllGathers (bf16) that overlap with remaining
    attention / o_proj compute.  The last head-pair's AllGather is split
    into token halves so the o_proj tail is not gated on one late
    collective.
  - host concatenates the 8 feature shards: no all-reduce needed.

v3: all-bf16 matmuls, Q resident in SBUF, causal mask on the DVE,
reciprocal_approx_fast, full-width RoPE, V transposed by the DMA
crossbar (no PE transposes), one shared 8-bank PSUM pool, and a
software-pipelined schedule: projection passes interleave attention
bb=0 (front-loaded so collectives fire early), o_proj bb=0 interleaves
attention bb=1, 3-deep score-buffer rotation keeps the PE dense in
attention-only stretches.
"""

import numpy as np

import concourse.bacc as bacc
import concourse.mybir as mybir
import concourse.tile as tile
from concourse.bass_utils import run_bass_kernel_spmd

F32 = mybir.dt.float32
F32R = mybir.dt.float32r
BF16 = mybir.dt.bfloat16
AF = mybir.ActivationFunctionType

N_CORES = 8
B, L = 2, 2048
N_HEADS, N_KV = 32, 8
HEAD_DIM = 128
D = N_HEADS * HEAD_DIM
THETA = 500000.0

EXP_BIAS = -8.0


def _rope_tables(t_all, l, dh):
    half = dh // 2
    inv = 1.0 / (THETA ** (np.arange(half, dtype=np.float64) * 2.0 / dh))
    pos = np.arange(t_all, dtype=np.float64) % l
    ang = inv[:, None] * pos[None, :]  # [half, T]
    cos = np.cos(ang)
    sin = np.sin(ang)
    cc = np.concatenate([cos, cos], 0).astype(np.float32)
    ss = np.concatenate([-sin, sin], 0).astype(np.float32)
    return cc, ss


def _build(n_cores=N_CORES, b=B, l=L, nh=N_HEADS, nkv=N_KV):
    dh = HEAD_DIM
    d = nh * dh
    t_all = b * l
    hpc = nh // n_cores  # q heads per core
    assert nkv == n_cores, "one kv head per core"
    mpc = d // n_cores  # o_proj output features per core
    kt_d = d // dh  # contraction tiles for projections
    ktl = l // 128  # key tiles per batch
    qg_n = l // 512  # 512-wide query groups per (batch, head)
    tg_n = t_all // 512  # 512-wide token groups for projections
    ksub = 4  # k-tiles per x subslab load
    assert kt_d % ksub == 0
    nsub = kt_d // ksub
    n_hp = hpc // 2  # head-pairs per core
    scale = dh ** -0.5

    nc = bacc.Bacc(
        "TRN2", target_bir_lowering=False, debug=False, num_devices=n_cores
    )

    xT = nc.dram_tensor("xT", [d, t_all], BF16, kind="ExternalInput").ap()
    wqT = nc.dram_tensor("wqT", [d, hpc * dh], BF16, kind="ExternalInput").ap()
    wkT = nc.dram_tensor("wkT", [d, dh], BF16, kind="ExternalInput").ap()
    wvT = nc.dram_tensor("wvT", [d, dh], BF16, kind="ExternalInput").ap()
    woT = nc.dram_tensor("woT", [d, mpc], BF16, kind="ExternalInput").ap()
    outT = nc.dram_tensor("outT", [mpc, t_all], F32, kind="ExternalOutput").ap()

    # compile-time constants
    cc_np, ss_np = _rope_tables(t_all, l, dh)
    mask_np = np.zeros((128, 4 * 512), dtype=np.float32)
    for j in range(4):
        k_idx = np.arange(128)[:, None]
        q_idx = np.arange(512)[None, :]
        mask_np[:, j * 512 : (j + 1) * 512] = (128 * j + k_idx <= q_idx).astype(
            np.float32
        )
    import ml_dtypes as _mld

    cc_c = nc.inline_tensor(cc_np, name="cc_c").ap()
    ss_c = nc.inline_tensor(ss_np, name="ss_c").ap()
    mask_c = nc.inline_tensor(
        mask_np.astype(_mld.bfloat16), name="mask_c"
    ).ap()
    ones_f_c = nc.inline_tensor(
        np.ones((128, 128), dtype=_mld.bfloat16), name="ones_f_c"
    ).ap()
    ident_c = nc.inline_tensor(np.eye(128, dtype=np.float32), name="ident_c").ap()

    with tile.TileContext(nc) as tc:
        with (
            tc.tile_pool(name="constp", bufs=1) as constp,
            tc.tile_pool(name="kvp", bufs=1) as kvp,
            tc.tile_pool(name="dramp", bufs=1, space="DRAM") as dramp,
            tc.tile_pool(name="mainps", bufs=1, space="PSUM") as mainps,
            tc.tile_pool(name="ppool", bufs=3) as ppool,
            tc.tile_pool(name="accp", bufs=2) as accp,
            tc.tile_pool(name="obf", bufs=2) as obf,
            tc.tile_pool(name="bsb", bufs=2) as bsb,
        ):
            bias_t = constp.tile([128, 1], F32, tag="bias_t")
            nc.vector.memset(bias_t[:], EXP_BIAS)
            masks = constp.tile([128, 4 * 512], BF16, tag="masks")
            ones_f = constp.tile([128, 128], BF16, tag="ones_f")

            K = kvp.tile([128, t_all], BF16, tag="Kres")  # rotated K^T
            Q = kvp.tile([128, hpc, t_all], BF16, tag="Qres")  # rotated Q^T
            Vn = kvp.tile([128, b, ktl, 128], BF16, tag="Vn")  # V

            # bounce / gathered DRAM tiles.  (1,1) is split into token
            # halves so its AllGathers can fire before all of bb=1's
            # attention is done.
            bounce = {}
            gathered = {}
            for bb in range(b):
                for hp in range(n_hp):
                    if bb == 1 and hp == 1:
                        continue
                    bounce[(bb, hp)] = dramp.tile(
                        [2 * dh, l], BF16, tag=f"bounce{bb}_{hp}",
                        name=f"bounce{bb}_{hp}",
                    )
                    gathered[(bb, hp)] = dramp.tile(
                        [n_cores * 2 * dh, l], BF16,
                        addr_space="Shared" if n_cores > 4 else "Local",
                        tag=f"gath{bb}_{hp}", name=f"gath{bb}_{hp}",
                    )
            for q in range(qg_n):
                bounce[(1, 1, q)] = dramp.tile(
                    [2 * dh, 512], BF16, tag=f"bounce11_{q}",
                    name=f"bounce11_{q}",
                )
                gathered[(1, 1, q)] = dramp.tile(
                    [n_cores * 2 * dh, 512], BF16,
                    addr_space="Shared" if n_cores > 4 else "Local",
                    tag=f"gath11_{q}", name=f"gath11_{q}",
                )

            wq_r = wqT.rearrange("(k p) m -> p k m", p=128)
            wk_r = wkT.rearrange("(k p) m -> p k m", p=128)
            wv_r = wvT.rearrange("(k p) m -> p k m", p=128)
            xT_r = xT.rearrange("(k p) t -> p k t", p=128)

            # PSUM tags: ppA0/ppA1 (projection passes / o_proj / V transp),
            # psp x2 [128,1024] (score pairs + borrowed softmax tail),
            # po x2 = 8 banks.
            def _ps(tag):
                bufs = 2 if tag in ("psp", "po") else 1
                width = 1024 if tag == "psp" else 512
                return mainps.tile([128, width], F32, tag=tag, name=tag, bufs=bufs)

            # ---------------- attention group ----------------------------

            def _bounce_dma(bb, h, g, ob):
                if bb == 1 and h >= 2:
                    tgt = bounce[(1, 1, g)]
                    nc.sync.dma_start(
                        tgt[(h % 2) * dh : (h % 2 + 1) * dh, :],
                        ob[:],
                    )
                else:
                    nc.sync.dma_start(
                        bounce[(bb, h // 2)][
                            (h % 2) * dh : (h % 2 + 1) * dh,
                            g * 512 : (g + 1) * 512,
                        ],
                        ob[:],
                    )

            def _attn_group_body(bb, h, g, psp_cycle=None):
                # key tiles processed in PAIRS: scores land in a 2-bank
                # [128,1024] PSUM tile, one exp covers both; PV is
                # pipelined one pair behind the scores.
                qoff = bb * l + g * 512
                nkt = 4 * g + 4
                po = _ps("po")
                acc = accp.tile([128, 512], F32, tag="acc", name="acc")
                acc_r = accp.tile([128, 512], BF16, tag="acc_r", name="acc_r")
                Ps = []  # P tile per pair
                for pr in range(nkt // 2):
                    psp = _ps("psp")
                    for half in range(2):
                        kt = 2 * pr + half
                        nc.tensor.matmul(
                            psp[:, half * 512 : (half + 1) * 512],
                            K[:, bb * l + kt * 128 : bb * l + (kt + 1) * 128],
                            Q[:, h, qoff : qoff + 512],
                            start=True,
                            stop=True,
                            skip_group_check=True,
                        )
                    P = ppool.tile([128, 1024], BF16, tag="P", name="P")
                    nc.scalar.activation(
                        P[:], psp[:], AF.Exp, scale=scale, bias=bias_t[:]
                    )
                    for half in range(2):
                        kt = 2 * pr + half
                        j = kt - 4 * g
                        if j >= 0:
                            nc.vector.tensor_mul(
                                P[:, half * 512 : (half + 1) * 512],
                                P[:, half * 512 : (half + 1) * 512],
                                masks[:, j * 512 : (j + 1) * 512],
                            )
                    if pr >= 1:
                        for half in range(2):
                            kt = 2 * (pr - 1) + half
                            nc.tensor.matmul(
                                po[:],
                                Vn[:, bb, kt, :],
                                Ps[pr - 1][:, half * 512 : (half + 1) * 512],
                                start=(kt == 0),
                                stop=False,
                                skip_group_check=True,
                            )
                    # denominator accumulation on the DVE
                    if pr == 0:
                        nc.vector.tensor_add(
                            acc[:], P[:, 0:512], P[:, 512:1024]
                        )
                    else:
                        nc.vector.tensor_add(acc[:], acc[:], P[:, 0:512])
                        if pr == nkt // 2 - 1:
                            nc.vector.tensor_add(
                                acc_r[:], acc[:], P[:, 512:1024]
                            )
                        else:
                            nc.vector.tensor_add(acc[:], acc[:], P[:, 512:1024])
                    Ps.append(P)
                for half in range(2):
                    kt = nkt - 2 + half
                    nc.tensor.matmul(
                        po[:],
                        Vn[:, bb, kt, :],
                        Ps[-1][:, half * 512 : (half + 1) * 512],
                        start=(kt == 0),
                        stop=(half == 1),
                        skip_group_check=True,
                    )

                def _tail():
                    # borrow half a scores-rotation PSUM tile for the
                    # broadcasted column sums
                    pm = _ps("psp")[:, 0:512]
                    nc.tensor.matmul(
                        pm, ones_f[:], acc_r[:],
                        start=True, stop=True, skip_group_check=True,
                    )
                    bs = bsb.tile([128, 512], F32, tag="bs", name="bs")
                    nc.vector.reciprocal_approx_fast(bs[:], pm)
                    ob = obf.tile([128, 512], BF16, tag="ob", name="ob")
                    nc.vector.tensor_mul(ob[:], po[:], bs[:])
                    _bounce_dma(bb, h, g, ob)

                return _tail

            def _fire_allgather(key):
                nc.gpsimd.collective_compute(
                    "AllGather",
                    mybir.AluOpType.bypass,
                    replica_groups=[list(range(n_cores))],
                    ins=[bounce[key].opt()],
                    outs=[gathered[key].opt()],
                )

            # ============ W1/W2: projections + attention bb=0 =============
            with (
                tc.tile_pool(name="wpool", bufs=1) as wpool,
                tc.tile_pool(name="xpool", bufs=1) as xpool,
                tc.tile_pool(name="ropet", bufs=2) as ropet,
            ):
                wq_sb = wpool.tile([128, kt_d, hpc * dh], BF16, tag="wq")
                wk_sb = wpool.tile([128, kt_d, dh], BF16, tag="wk")
                wv_sb = wpool.tile([128, kt_d, dh], BF16, tag="wv")
                cc_sb = wpool.tile([128, t_all], F32, tag="cc_sb")
                ss_sb = wpool.tile([128, t_all], F32, tag="ss_sb")
                VT = wpool.tile([128, t_all], F32, tag="VTres")  # V^T
                ident = wpool.tile([128, 128], F32, tag="ident")


                def _vn_batch(bb):
                    for kt in range(ktl):
                        pt = _ps("ppA0" if kt % 2 == 0 else "ppA1")
                        nc.tensor.transpose(
                            pt[0:128, 0:128],
                            VT[:, bb * l + kt * 128 : bb * l + (kt + 1) * 128],
                            ident[:],
                        )
                        nc.scalar.activation(
                            Vn[:, bb, kt, :], pt[0:128, 0:128], AF.Copy
                        )

                def _rope(dst, psrc, toff):
                    ts_t = ropet.tile([128, 512], F32, tag="ts")
                    r1 = ropet.tile([128, 512], F32, tag="r1")
                    nc.scalar.activation(ts_t[0:64, :], psrc[64:128, :], AF.Copy)
                    nc.scalar.activation(ts_t[64:128, :], psrc[0:64, :], AF.Copy)
                    nc.vector.tensor_mul(r1[:], psrc[:], cc_sb[:, toff : toff + 512])
                    nc.vector.tensor_mul(ts_t[:], ts_t[:], ss_sb[:, toff : toff + 512])
                    nc.vector.tensor_add(dst, r1[:], ts_t[:])

                first_load = [True]
                xs_cur = {}

                def _xs_for(tg):
                    # one resident x slab per token group (two half-tiles so
                    # the next group's first half can load while the last
                    # pass still reads the second); x read from HBM once.
                    if tg in xs_cur:
                        return xs_cur[tg]
                    xs_cur.clear()
                    kh = kt_d // 2
                    halves = (
                        xpool.tile([128, kh, 512], BF16, tag="xsA", name="xsA"),
                        xpool.tile([128, kh, 512], BF16, tag="xsB", name="xsB"),
                    )
                    toff = tg * 512
                    for hf in range(2):
                        for sub in range(nsub // 2):
                            ks = slice(sub * ksub, (sub + 1) * ksub)
                            gs = slice(
                                hf * kh + sub * ksub, hf * kh + (sub + 1) * ksub
                            )
                            # split across two DMA queues where the ACT
                            # queue is light: all of W1, and W2's last two
                            # groups (attention exps exhausted by then)
                            dual = tg < tg_n // 2 or tg >= tg_n - 2
                            eng = (
                                nc.scalar
                                if (dual and sub % 2 == 1)
                                else nc.sync
                            )
                            eng.dma_start(
                                halves[hf][:, ks, :],
                                xT_r[:, gs, toff : toff + 512],
                            )
                    xs_cur[tg] = halves
                    return halves

                def _proj_pass(tg, otiles, tags):
                    toff = tg * 512
                    banks = [_ps(tags[i]) for i in range(len(otiles))]
                    xs = _xs_for(tg)
                    if first_load[0]:
                        for rs in (
                            slice(0, ksub),
                            slice(ksub, 3 * ksub),
                            slice(3 * ksub, kt_d),
                        ):
                            nc.gpsimd.dma_start(wq_sb[:, rs, :], wq_r[:, rs, :])
                            nc.gpsimd.dma_start(wk_sb[:, rs, :], wk_r[:, rs, :])
                            nc.gpsimd.dma_start(wv_sb[:, rs, :], wv_r[:, rs, :])
                        first_load[0] = False
                    for sub in range(nsub):
                        for i, ot in enumerate(otiles):
                            if ot[0] == "q":
                                w_ap = wq_sb
                                msl = slice(ot[1] * dh, (ot[1] + 1) * dh)
                            elif ot[0] == "k":
                                w_ap = wk_sb
                                msl = slice(0, dh)
                            else:
                                w_ap = wv_sb
                                msl = slice(0, dh)
                            for k in range(ksub):
                                kt = sub * ksub + k
                                kh = kt_d // 2
                                nc.tensor.matmul(
                                    banks[i][:],
                                    w_ap[:, kt, msl],
                                    xs[kt // kh][:, kt % kh, :],
                                    start=(kt == 0),
                                    stop=(kt == kt_d - 1),
                                    skip_group_check=True,
                                )
                    for i, ot in enumerate(otiles):
                        if ot[0] == "q":
                            _rope(Q[:, ot[1], toff : toff + 512], banks[i][:], toff)
                        elif ot[0] == "k":
                            _rope(K[:, toff : toff + 512], banks[i][:], toff)
                        else:
                            nc.scalar.activation(
                                VT[:, toff : toff + 512], banks[i][:], AF.Copy
                            )

                # prime the first token group's x chunks on both queues,
                # then let the constants ride the ACT DMA queue behind them
                _xs_for(0)
                nc.scalar.dma_start(cc_sb[:], cc_c)
                nc.scalar.dma_start(ss_sb[:], ss_c)
                nc.scalar.dma_start(masks[:], mask_c)
                nc.scalar.dma_start(ones_f[:], ones_f_c)
                nc.scalar.dma_start(ident[:], ident_c)

                PASSES = (
                    [("q", 0), ("q", 1)],
                    [("q", 2), ("q", 3)],
                    [("k",), ("v",)],
                )
                TAGS_A2 = ["ppA0", "ppA1"]
                TAGS_B2 = ["po", "po"]

                # ---- W1: tg 0..3 dense, ping-pong PSUM tag sets ----
                pidx = 0
                for tg in range(tg_n // 2):
                    for ot in PASSES:
                        _proj_pass(tg, ot, TAGS_A2 if pidx % 2 == 0 else TAGS_B2)
                        pidx += 1
                _vn_batch(0)

                # ---- W2: tg 4..7 + attention bb=0, front-loaded ----
                attn_iter = [(0, h, g) for h in range(hpc) for g in range(qg_n)]
                st = {"ai": 0, "tail": None}

                def _emit_attn_unit():
                    if st["ai"] >= len(attn_iter):
                        return
                    bb, h, g = attn_iter[st["ai"]]
                    st["ai"] += 1
                    tail = _attn_group_body(bb, h, g)
                    if st["tail"] is not None:
                        st["tail"]()
                    st["tail"] = tail
                    if g == qg_n - 1 and h % 2 == 1:
                        st["tail"]()
                        st["tail"] = None
                        _fire_allgather((bb, h // 2))

                pace = [3, 3, 2, 2, 2, 2, 1, 1, 0, 0, 0, 0]
                passes = []
                for tg in range(tg_n // 2, tg_n):
                    for ot in PASSES:
                        passes.append((tg, ot))
                for pi, (tg, otiles) in enumerate(passes):
                    # once attention units are exhausted, ping-pong with the
                    # po tags (free again) to decouple back-to-back passes
                    tags = TAGS_B2 if (pi >= 8 and pi % 2 == 1) else TAGS_A2
                    _proj_pass(tg, otiles, tags)
                    for _ in range(pace[pi]):
                        _emit_attn_unit()
                while st["ai"] < len(attn_iter):
                    _emit_attn_unit()
                if st["tail"] is not None:
                    st["tail"]()
                    st["tail"] = None
                _vn_batch(1)

            # ============ W3/W4: attention bb=1 + o_proj ==================
            with (
                tc.tile_pool(name="wopool", bufs=1) as wopool,
                tc.tile_pool(name="ogpool", bufs=2) as ogpool,
                tc.tile_pool(name="outst", bufs=3) as outst,
            ):
                wo_sb = wopool.tile([128, kt_d, mpc], BF16, tag="wo")
                wo_r = woT.rearrange("(k p) m -> p k m", p=128)
                # chunk 0 first (ungated) so the first o_proj m-block can
                # start as soon as its og lands; rest follow on gpsimd
                nc.gpsimd.dma_start(
                    wo_sb[:, :, 0:dh], wo_r[:, :, 0:dh]
                )

                kt_map = []
                for hp in range(n_hp):
                    for c in range(n_cores):
                        for hl in range(2):
                            kt_map.append(4 * c + 2 * hp + hl)
                blk = n_cores * 2

                g_rs = {}
                og_tiles = {}

                def _g_view(key):
                    if key not in g_rs:
                        g_rs[key] = gathered[key][:].rearrange(
                            "(k p) t -> p k t", p=128
                        )
                    return g_rs[key]

                def _og_load(bb, tgl):
                    og = ogpool.tile([128, kt_d, 512], BF16, tag="og", name="og")
                    nc.gpsimd.dma_start(
                        og[:, 0:blk, :],
                        _g_view((bb, 0))[:, :, tgl * 512 : (tgl + 1) * 512],
                    )
                    if bb == 1:
                        src = _g_view((1, 1, tgl))
                        nc.gpsimd.dma_start(og[:, blk : 2 * blk, :], src[:, :, :])
                    else:
                        nc.gpsimd.dma_start(
                            og[:, blk : 2 * blk, :],
                            _g_view((0, 1))[:, :, tgl * 512 : (tgl + 1) * 512],
                        )
                    return og

                oproj_units = [
                    (bb, tgl, m)
                    for bb in range(b)
                    for tgl in range(l // 512)
                    for m in range(mpc // 128)
                ]
                oi = [0]

                def _ensure_og(idx):
                    bb, tgl, m = oproj_units[idx]
                    key = (bb, tgl)
                    if key not in og_tiles:
                        og_tiles[key] = _og_load(bb, tgl)
                    return og_tiles[key]

                def _emit_oproj_unit():
                    if oi[0] >= len(oproj_units):
                        return False
                    bb, tgl, m = oproj_units[oi[0]]
                    og = _ensure_og(oi[0])
                    if m == 0 and oi[0] + 4 < len(oproj_units):
                        _ensure_og(oi[0] + 4)
                    pp = _ps("ppA0" if (oi[0] % 2 == 0) else "ppA1")
                    for kt in range(kt_d):
                        nc.tensor.matmul(
                            pp[:],
                            wo_sb[:, kt_map[kt], m * 128 : (m + 1) * 128],
                            og[:, kt, :],
                            start=(kt == 0),
                            stop=(kt == kt_d - 1),
                            skip_group_check=True,
                        )
                    ot = outst.tile([128, 512], F32, tag="ot", name="ot")
                    nc.scalar.activation(ot[:], pp[:], AF.Copy)
                    nc.sync.dma_start(
                        outT[
                            m * 128 : (m + 1) * 128,
                            bb * l + tgl * 512 : bb * l + (tgl + 1) * 512,
                        ],
                        ot[:],
                    )
                    oi[0] += 1
                    return True

                _ensure_og(0)
                _ensure_og(4)
                for m in range(1, mpc // dh):
                    nc.gpsimd.dma_start(
                        wo_sb[:, :, m * dh : (m + 1) * dh],
                        wo_r[:, :, m * dh : (m + 1) * dh],
                    )

                # bb=1 attention order: hp0's heads first (fires its full
                # AllGather), then hp1 g-major so its AllGathers fire per
                # 512-token quarter and the o_proj tail drains pipelined.
                attn_iter2 = (
                    [(1, 0, g) for g in range(qg_n)]
                    + [(1, 1, g) for g in range(qg_n)]
                    + [(1, h, g) for g in range(qg_n) for h in (2, 3)]
                )
                CYC3 = ["psp", "ppA2", "psp"]
                pending = None
                for u, (bb, h, g) in enumerate(attn_iter2):
                    tail = _attn_group_body(bb, h, g, CYC3)
                    if pending is not None:
                        pending()
                    pending = tail
                    if u == 7:
                        pending()
                        pending = None
                        _fire_allgather((1, 0))
                    elif u >= 9 and (u - 9) % 2 == 0:
                        pending()
                        pending = None
                        _fire_allgather((1, 1, (u - 9) // 2))
                    if u >= 4:
                        _emit_oproj_unit()
                    if u >= 12:
                        _emit_oproj_unit()
                if pending is not None:
                    pending()
                while _emit_oproj_unit():
                    pass

    nc.compile()
    return nc


_NC_CACHE = {}


def _get_nc(key=(N_CORES, B, L, N_HEADS, N_KV)):
    if key not in _NC_CACHE:
        _NC_CACHE[key] = _build(*key)
    return _NC_CACHE[key]


def make_in_maps(x, Wq, Wk, Wv, Wo, n_cores=N_CORES):
    import ml_dtypes

    bf16 = ml_dtypes.bfloat16
    b, l, d = x.shape
    nh = Wq.shape[0] // HEAD_DIM
    hpc = nh // n_cores
    mpc = d // n_cores
    xT = np.ascontiguousarray(x.reshape(b * l, d).T.astype(bf16))
    in_maps = []
    for c in range(n_cores):
        wq_c = np.ascontiguousarray(
            Wq[c * hpc * HEAD_DIM : (c + 1) * hpc * HEAD_DIM, :].T.astype(bf16)
        )
        wk_c = np.ascontiguousarray(
            Wk[c * HEAD_DIM : (c + 1) * HEAD_DIM, :].T.astype(bf16)
        )
        wv_c = np.ascontiguousarray(
            Wv[c * HEAD_DIM : (c + 1) * HEAD_DIM, :].T.astype(bf16)
        )
        wo_c = np.ascontiguousarray(
            Wo[c * mpc : (c + 1) * mpc, :].T.astype(bf16)
        )
        in_maps.append(
            {"xT": xT, "wqT": wq_c, "wkT": wk_c, "wvT": wv_c, "woT": wo_c}
        )
    return in_maps


def assemble_out(results, b, l, d):
    parts = [r["outT"] for r in results]
    outT = np.concatenate(parts, axis=0)  # [D, T]
    return np.ascontiguousarray(outT.T).reshape(b, l, d).astype(np.float32)


def kernel(x, Wq, Wk, Wv, Wo, trace=False):
    x = np.asarray(x, dtype=np.float32)
    nc = _get_nc()
    in_maps = make_in_maps(x, Wq, Wk, Wv, Wo)
    res = run_bass_kernel_spmd(nc, in_maps, list(range(N_CORES)), trace=trace)
    out = assemble_out(res.results, *x.shape)
    if trace:
        return out, res
    return out


if __name__ == "__main__":
    rng = np.random.default_rng(0)
    s = 0.02
    x = rng.standard_normal((B, L, D)).astype(np.float32)
    Wq = (rng.standard_normal((D, D)) * s).astype(np.float32)
    Wk = (rng.standard_normal((N_KV * HEAD_DIM, D)) * s).astype(np.float32)
    Wv = (rng.standard_normal((N_KV * HEAD_DIM, D)) * s).astype(np.float32)
    Wo = (rng.standard_normal((D, D)) * s).astype(np.float32)
    out = kernel(x, Wq, Wk, Wv, Wo)
    print(out.shape, out.dtype)
